# revision 8
# baseline (speedup 1.0000x reference)
"""DNDT (deep neural decision tree) forward kernel for 8 Trainium2 NeuronCores.

Math (per batch row b of 16384):
  h[f,j]   = (x[b,f] * W[j] + bias[f,j]) / t,  W = [1..4], bias = cumsum([0,-sorted_cuts])
  bins     = softmax_j(h)                       # [6, 4]
  leaf     = kron(bins[0], ..., bins[5])        # [4096]
  out[b]   = leaf @ leaf_score                  # [10]

Device algorithm (pure data parallel, 2048 rows/core, batch-major layout
[128 partitions x 16 rows-per-partition], fp16 datapath after the exp):
  * softmax shift g(x) = (x + 3*relu(x))/t keeps every exponent <= 0, so
    E = exp(h - g) never overflows; exp runs on the scalar (ACT) engine.
  * bins are normalized PER FEATURE (En = E / sum_j E) right after the exp:
    all kron products then live in [0,1], which makes the fp16 pipeline safe
    (no giant normalizer reciprocal at the end).
  * leaf is factored 4+2: A = p01 (x) p23 (256-wide) on DVE, p45 (16-wide);
      C[b,(c,v)] = A @ S2,  S2[u,(c,v)] = leaf_score[u*16+v, c]   (PE, fp16)
      out[b,c]   = sum_v C[b,c,v] * p45[b,v]                      (DVE)
  * the A kron is built with a duplicated outer factor p01x[...,2] so every
    operand has a packed 2-byte last dim -> DVE 2x fast mode.
  * A is transposed 128x128 at a time on the PE (fp16 transposes keep their
    dtype in PSUM); the PSUM->SBUF bounce copies are batched 4 chunks at a
    time and spread across DVE/ACT/Pool.  C is copied out of PSUM by ACT in
    fp16 so the final v-contraction (mult + add-tree) runs in DVE 2x mode.
  * H-stage adds run on the gpsimd (Pool) engine; a junk fp32 matmul warms
    the PE HAM clock gate while the front runs.
"""

import numpy as np

import concourse.bass as bass
import concourse.tile as tile
from concourse import bacc, mybir
from concourse.bass_utils import run_bass_kernel_spmd

N_CORES = 8
B = 16384
BC = B // N_CORES          # rows per core = 2048
P = 128                    # partitions
M = BC // P                # rows per partition = 16
NSC = 2                    # super-chunks (pipeline stages)
SCM = M // NSC             # rows per partition per super-chunk = 8
F32 = mybir.dt.float32
F16 = mybir.dt.float16
AX = mybir.AxisListType
OP = mybir.AluOpType


def _build_nc(invt):
    neg3invt = -3.0 * invt
    nc = bacc.Bacc("TRN2", target_bir_lowering=False, debug=False,
                   num_devices=N_CORES)
    xd = nc.dram_tensor("x", [P, M * 6], F32, kind="ExternalInput")
    cstd = nc.dram_tensor("cst", [P, 48], F32, kind="ExternalInput")
    s2d = nc.dram_tensor("s2", [256, 160], F16, kind="ExternalInput")
    idd = nc.dram_tensor("ident", [P, P], F16, kind="ExternalInput")
    od = nc.dram_tensor("o", [P, M * 10], F32, kind="ExternalOutput")

    with tile.TileContext(nc) as tc:
        with tc.tile_pool(name="consts", bufs=1) as consts, \
             tc.tile_pool(name="work", bufs=2) as work, \
             tc.tile_pool(name="atp", bufs=4) as atp, \
             tc.tile_pool(name="ps_t", bufs=4, space="PSUM") as ps_t, \
             tc.tile_pool(name="ps_c", bufs=2, space="PSUM") as ps_c:
            x_st = consts.tile([P, M, 6], F32)
            nc.sync.dma_start(out=x_st[:],
                              in_=xd[:].rearrange("p (i f) -> p i f", i=M))
            cst_st = consts.tile([P, 2, 6, 4], F32)
            nc.sync.dma_start(out=cst_st[:].rearrange("p k f j -> p (k f j)"),
                              in_=cstd[:])
            s2_sb = consts.tile([P, 2, 160], F16)
            nc.sync.dma_start(out=s2_sb[:],
                              in_=s2d[:].rearrange("(k p) n -> p k n", p=P))
            ident = consts.tile([P, P], F16)
            nc.sync.dma_start(out=ident[:], in_=idd[:])

            # HAM / p-state warm-up: junk fp32 matmuls on the const tile keep
            # the PE clock ramping while the DVE/Pool front runs.  They write
            # into the first C psum buffer, which is reused afterwards.
            warm_ps = ps_c.tile([P, 4, 256], F32, tag="cp")
            cw = cst_st[:].rearrange("p k f j -> p (k f j)")
            for w in range(12):
                nc.tensor.matmul(warm_ps[0:48, w % 4, 0:48], lhsT=cw, rhs=cw,
                                 start=True, stop=True)

            # ---- per-super-chunk front: x -> En (normalized bins, fp16) ----
            def front(sc):
                xv = x_st[:, sc * SCM:(sc + 1) * SCM, :]
                r2 = work.tile([P, SCM, 6, 1], F32, tag="r2")
                nc.vector.tensor_scalar(out=r2[:, :, :, 0], in0=xv,
                                        scalar1=0.0, scalar2=neg3invt,
                                        op0=OP.max, op1=OP.mult)
                Hm = work.tile([P, SCM, 6, 4], F32, tag="Hm")
                nc.vector.tensor_mul(
                    Hm[:], xv[:, :, :, None].broadcast_to((P, SCM, 6, 4)),
                    cst_st[:, 0:1, :, :].broadcast_to((P, SCM, 6, 4)))
                H1 = work.tile([P, SCM, 6, 4], F32, tag="H1")
                nc.gpsimd.tensor_add(
                    H1[:], Hm[:],
                    cst_st[:, 1:2, :, :].broadcast_to((P, SCM, 6, 4)))
                H = work.tile([P, SCM, 6, 4], F32, tag="H")
                nc.gpsimd.tensor_add(H[:], H1[:],
                                     r2[:].broadcast_to((P, SCM, 6, 4)))
                E = work.tile([P, SCM, 6, 4], F32, tag="E")
                nc.scalar.activation(E[:].rearrange("p i f j -> p (i f j)"),
                                     H[:].rearrange("p i f j -> p (i f j)"),
                                     mybir.ActivationFunctionType.Exp)
                Z = work.tile([P, SCM, 6], F32, tag="Z")
                nc.vector.tensor_reduce(Z[:], E[:], axis=AX.X, op=OP.add)
                zrf = work.tile([P, SCM, 6], F32, tag="zrf")
                nc.vector.reciprocal(zrf[:], Z[:])
                En = work.tile([P, SCM, 6, 4], F16, tag="En")
                nc.vector.tensor_mul(
                    En[:], E[:], zrf[:, :, :, None].broadcast_to((P, SCM, 6, 4)))

                # small krons (fp16).  p23 is stored duplicated along a x2
                # last dim so the A kron can run with packed 2-byte last dims
                # on every operand (DVE 2x mode) within the ISA 3D AP limit.
                p01 = work.tile([P, SCM, 16], F16, tag="p01")
                nc.vector.tensor_mul(
                    p01[:].rearrange("p i (a b) -> p i a b", a=4),
                    En[:, :, 0, :, None].broadcast_to((P, SCM, 4, 4)),
                    En[:, :, 1, None, :].broadcast_to((P, SCM, 4, 4)))
                p23x = work.tile([P, SCM, 16, 2], F16, tag="p23x")
                for d in range(2):
                    nc.gpsimd.tensor_mul(
                        p23x[:, :, :, d].rearrange("p i (a b) -> p i a b", a=4),
                        En[:, :, 2, :, None].broadcast_to((P, SCM, 4, 4)),
                        En[:, :, 3, None, :].broadcast_to((P, SCM, 4, 4)))
                p45 = work.tile([P, SCM, 16], F16, tag="p45")
                nc.gpsimd.tensor_mul(
                    p45[:].rearrange("p i (a b) -> p i a b", a=4),
                    En[:, :, 4, :, None].broadcast_to((P, SCM, 4, 4)),
                    En[:, :, 5, None, :].broadcast_to((P, SCM, 4, 4)))

                # A[i, b, a] = p01[i, a] * p23[i, b]   (p23-major memory
                # order; fixed up by a host-side permutation of S2 rows).
                # 8 strip instructions, each [i, b, 2] with packed last dims.
                A = work.tile([P, SCM, 16, 16], F16, tag="A")
                for ah in range(8):
                    eng = nc.vector if ah < 6 else nc.gpsimd
                    eng.tensor_mul(
                        A[:, :, :, 2 * ah:2 * ah + 2],
                        p01[:, :, None, 2 * ah:2 * ah + 2].broadcast_to(
                            (P, SCM, 16, 2)),
                        p23x[:])
                return A, p45

            # ---- per-super-chunk back: A -> C -> out rows ----
            def back(sc, A, p45):
                O = work.tile([P, SCM, 10], F32, tag="O")
                Ds = work.tile([P, SCM, 10, 16], F16, tag="Ds")
                Af = A[:].rearrange("p i b a -> p i (b a)")
                for g in range(2):          # groups of 4 chunks
                    g4 = sc * 2 + g
                    tp = ps_t.tile([P, 8, P], F16, tag="tp")
                    for j in range(4):
                        i = g * 4 + j
                        for k in range(2):
                            nc.tensor.transpose(tp[:, j * 2 + k, :],
                                                Af[:, i, k * P:(k + 1) * P],
                                                ident[:])
                    at2 = atp.tile([P, 8, P], F16, tag="at")
                    if g4 == 0 or g4 == 3:
                        nc.vector.tensor_copy(out=at2[:], in_=tp[:])
                    else:
                        nc.scalar.copy(out=at2[:], in_=tp[:])
                    cpp = ps_c.tile([P, 4, 256], F32, tag="cp")
                    for j in range(4):
                        for k in range(2):
                            nc.tensor.matmul(cpp[:, j, 0:160],
                                             lhsT=at2[:, j * 2 + k, :],
                                             rhs=s2_sb[:, k, :],
                                             start=(k == 0), stop=(k == 1))
                    Cs = work.tile([P, 4, 10, 16], F16, tag=f"Cs{g}")
                    nc.scalar.copy(
                        out=Cs[:].rearrange("p j c v -> p j (c v)"),
                        in_=cpp[:, :, 0:160])
                    sl = slice(g * 4, (g + 1) * 4)
                    nc.vector.tensor_mul(
                        Ds[:, sl], Cs[:],
                        p45[:, sl, None, :].broadcast_to((P, 4, 10, 16)))
                # v-contraction as an add tree (fp16 2x until the last level)
                T1 = work.tile([P, SCM, 10, 8], F16, tag="T1")
                nc.vector.tensor_add(T1[:], Ds[:, :, :, 0:8], Ds[:, :, :, 8:16])
                T2 = work.tile([P, SCM, 10, 4], F16, tag="T2")
                nc.vector.tensor_add(T2[:], T1[:, :, :, 0:4], T1[:, :, :, 4:8])
                T3 = work.tile([P, SCM, 10, 2], F16, tag="T3")
                nc.vector.tensor_add(T3[:], T2[:, :, :, 0:2], T2[:, :, :, 2:4])
                nc.vector.tensor_add(O[:], T3[:, :, :, 0], T3[:, :, :, 1])
                nc.sync.dma_start(
                    out=od[:].rearrange("p (i c) -> p i c", i=M)[
                        :, sc * SCM:(sc + 1) * SCM, :],
                    in_=O[:])

            A0, p450 = front(0)
            A1, p451 = front(1)
            back(0, A0, p450)
            back(1, A1, p451)
    nc.compile()
    return nc


_CACHE = {}


def _host_prep(x, cuts, leaf_score, temperature):
    x = np.ascontiguousarray(np.asarray(x, dtype=np.float32))
    cuts = np.asarray(cuts, dtype=np.float32)
    leaf_score = np.asarray(leaf_score, dtype=np.float32)
    invt = 1.0 / float(np.asarray(temperature).reshape(-1)[0])

    sc = np.sort(cuts, axis=1)
    bias = np.cumsum(np.concatenate([np.zeros((6, 1), np.float32), -sc],
                                    axis=1, dtype=np.float32), axis=1)  # [6,4]
    W = np.arange(1.0, 5.0, dtype=np.float32)
    w2 = np.tile(((W - 1.0) * invt)[None, :], (6, 1))                   # [6,4]
    bt = bias * invt                                                    # [6,4]
    cst = np.ascontiguousarray(np.broadcast_to(
        np.stack([w2, bt]).reshape(1, 48), (P, 48)).astype(np.float32))
    # S2[w,(c,v)] = leaf_score[w*16+v, c], then permute rows a*16+b -> b*16+a
    # to match the device A[i, b, a] memory order.
    s2 = (leaf_score.reshape(256, 16, 10).transpose(0, 2, 1)
          .reshape(16, 16, 160).transpose(1, 0, 2).reshape(256, 160))
    s2 = np.ascontiguousarray(s2.astype(np.float16))
    ident = np.eye(P, dtype=np.float16)

    xs = x.reshape(N_CORES, P, M * 6)
    in_maps = [{"x": xs[i], "cst": cst, "s2": s2, "ident": ident}
               for i in range(N_CORES)]
    return invt, in_maps


def kernel(x, cuts, leaf_score, temperature):
    invt, in_maps = _host_prep(x, cuts, leaf_score, temperature)
    key = ("nc", float(invt))
    if key not in _CACHE:
        _CACHE[key] = _build_nc(invt)
        _CACHE["nc"] = _CACHE[key]
    nc = _CACHE[key]
    res = run_bass_kernel_spmd(nc, in_maps, list(range(N_CORES))).results
    out = np.concatenate([r["o"].reshape(BC, 10) for r in res], axis=0)
    return out.astype(np.float32)


# revision 9
# speedup vs baseline: 1.2011x; 1.2011x over previous
"""DNDT (deep neural decision tree) forward kernel for 8 Trainium2 NeuronCores.

Math (per batch row b of 16384):
  h[f,j]   = (x[b,f] * W[j] + bias[f,j]) / t,  W = [1..4], bias = cumsum([0,-sorted_cuts])
  bins     = softmax_j(h)                       # [6, 4]
  leaf     = kron(bins[0], ..., bins[5])        # [4096]
  out[b]   = leaf @ leaf_score                  # [10]

Device algorithm (pure data parallel, 2048 rows/core, batch-major layout
[128 partitions x 16 rows-per-partition], fp16 datapath after the exp):
  * softmax shift g(x) = (x + 3*relu(x))/t keeps every exponent <= 0, so
    E = exp(h - g) never overflows; exp runs on the scalar (ACT) engine.
  * bins are normalized PER FEATURE (En = E / sum_j E) right after the exp:
    all kron products then live in [0,1], which makes the fp16 pipeline safe
    (no giant normalizer reciprocal at the end).
  * leaf is factored 4+2: A = p01 (x) p23 (256-wide, [i, b, a] memory order
    with a host-side S2 row permutation), p45 (16-wide);
      C[b,(c,v)] = A @ S2,  S2[u,(c,v)] = leaf_score[u*16+v, c]   (PE, fp16)
      out[b,c]   = sum_v C[b,c,v] * p45[b,v]            (DVE mult + add tree)
  * the A kron runs as 8 strip instructions with a x2-duplicated p23 so every
    operand has a packed 2-byte last dim (DVE 2x mode, ISA 3D AP limit).
  * A is transposed for the matmul by the DMA XBAR (dma_start_transpose,
    SBUF->SBUF, 2-byte) - no PE transposes and no PSUM bounce copies.
  * C is copied out of PSUM by ACT in fp16 so the final v-contraction runs in
    DVE 2x mode.  Junk fp32 matmuls warm the PE clock gate during the front.
"""

import numpy as np

import concourse.bass as bass
import concourse.tile as tile
from concourse import bacc, mybir
from concourse.bass_utils import run_bass_kernel_spmd

N_CORES = 8
B = 16384
BC = B // N_CORES          # rows per core = 2048
P = 128                    # partitions
M = BC // P                # rows per partition = 16
NSC = 2                    # super-chunks (pipeline stages)
SCM = M // NSC             # rows per partition per super-chunk = 8
F32 = mybir.dt.float32
F16 = mybir.dt.float16
AX = mybir.AxisListType
OP = mybir.AluOpType


def _build_nc(invt):
    neg3invt = -3.0 * invt
    nc = bacc.Bacc("TRN2", target_bir_lowering=False, debug=False,
                   num_devices=N_CORES)
    xd = nc.dram_tensor("x", [P, M * 6], F32, kind="ExternalInput")
    cstd = nc.dram_tensor("cst", [P, 48], F32, kind="ExternalInput")
    s2d = nc.dram_tensor("s2", [256, 160], F16, kind="ExternalInput")
    od = nc.dram_tensor("o", [P, M * 10], F32, kind="ExternalOutput")

    with tile.TileContext(nc) as tc:
        with tc.tile_pool(name="consts", bufs=1) as consts, \
             tc.tile_pool(name="work", bufs=2) as work, \
             tc.tile_pool(name="atp", bufs=4) as atp, \
             tc.tile_pool(name="ps_c", bufs=3, space="PSUM") as ps_c, \
             tc.tile_pool(name="ps_w", bufs=1, space="PSUM") as ps_w:
            x_st = consts.tile([P, M, 6], F32)
            nc.sync.dma_start(out=x_st[:],
                              in_=xd[:].rearrange("p (i f) -> p i f", i=M))
            cst_st = consts.tile([P, 2, 6, 4], F32)
            nc.sync.dma_start(out=cst_st[:].rearrange("p k f j -> p (k f j)"),
                              in_=cstd[:])
            s2_sb = consts.tile([P, 2, 160], F16)
            nc.sync.dma_start(out=s2_sb[:],
                              in_=s2d[:].rearrange("(k p) n -> p k n", p=P))

            # HAM / p-state warm-up: junk fp32 matmuls on the const tile keep
            # the PE clock ramping while the DVE/Pool front runs.
            warm_ps = ps_w.tile([P, 256], F32)
            cw = cst_st[:].rearrange("p k f j -> p (k f j)")
            for w in range(10):
                nc.tensor.matmul(warm_ps[0:48, 0:48], lhsT=cw, rhs=cw,
                                 start=True, stop=True)

            # ---- per-super-chunk front: x -> En (normalized bins, fp16) ----
            def front(sc):
                xv = x_st[:, sc * SCM:(sc + 1) * SCM, :]
                r2 = work.tile([P, SCM, 6, 1], F32, tag="r2")
                nc.vector.tensor_scalar(out=r2[:, :, :, 0], in0=xv,
                                        scalar1=0.0, scalar2=neg3invt,
                                        op0=OP.max, op1=OP.mult)
                Hm = work.tile([P, SCM, 6, 4], F32, tag="Hm")
                nc.vector.tensor_mul(
                    Hm[:], xv[:, :, :, None].broadcast_to((P, SCM, 6, 4)),
                    cst_st[:, 0:1, :, :].broadcast_to((P, SCM, 6, 4)))
                H1 = work.tile([P, SCM, 6, 4], F32, tag="H1")
                nc.gpsimd.tensor_add(
                    H1[:], Hm[:],
                    cst_st[:, 1:2, :, :].broadcast_to((P, SCM, 6, 4)))
                H = work.tile([P, SCM, 6, 4], F32, tag="H")
                nc.gpsimd.tensor_add(H[:], H1[:],
                                     r2[:].broadcast_to((P, SCM, 6, 4)))
                E = work.tile([P, SCM, 6, 4], F32, tag="E")
                nc.scalar.activation(E[:].rearrange("p i f j -> p (i f j)"),
                                     H[:].rearrange("p i f j -> p (i f j)"),
                                     mybir.ActivationFunctionType.Exp)
                Z = work.tile([P, SCM, 6], F32, tag="Z")
                nc.vector.tensor_reduce(Z[:], E[:], axis=AX.X, op=OP.add)
                zrf = work.tile([P, SCM, 6], F32, tag="zrf")
                nc.vector.reciprocal(zrf[:], Z[:])
                En = work.tile([P, SCM, 6, 4], F16, tag="En")
                nc.vector.tensor_mul(
                    En[:], E[:], zrf[:, :, :, None].broadcast_to((P, SCM, 6, 4)))

                # small krons (fp16).  p23 is stored duplicated along a x2
                # last dim so the A kron can run with packed 2-byte last dims
                # on every operand (DVE 2x mode) within the ISA 3D AP limit.
                p01 = work.tile([P, SCM, 16], F16, tag="p01")
                nc.vector.tensor_mul(
                    p01[:].rearrange("p i (a b) -> p i a b", a=4),
                    En[:, :, 0, :, None].broadcast_to((P, SCM, 4, 4)),
                    En[:, :, 1, None, :].broadcast_to((P, SCM, 4, 4)))
                p23x = work.tile([P, SCM, 16, 2], F16, tag="p23x")
                for d in range(2):
                    nc.gpsimd.tensor_mul(
                        p23x[:, :, :, d].rearrange("p i (a b) -> p i a b", a=4),
                        En[:, :, 2, :, None].broadcast_to((P, SCM, 4, 4)),
                        En[:, :, 3, None, :].broadcast_to((P, SCM, 4, 4)))
                p45 = work.tile([P, SCM, 16], F16, tag="p45")
                nc.gpsimd.tensor_mul(
                    p45[:].rearrange("p i (a b) -> p i a b", a=4),
                    En[:, :, 4, :, None].broadcast_to((P, SCM, 4, 4)),
                    En[:, :, 5, None, :].broadcast_to((P, SCM, 4, 4)))

                # A[i, b, a] = p01[i, a] * p23[i, b]   (p23-major memory
                # order; fixed up by a host-side permutation of S2 rows).
                # 8 strip instructions, each [i, b, 2] with packed last dims.
                A = work.tile([P, SCM, 16, 16], F16, tag="A")
                for ah in range(8):
                    nc.vector.tensor_mul(
                        A[:, :, :, 2 * ah:2 * ah + 2],
                        p01[:, :, None, 2 * ah:2 * ah + 2].broadcast_to(
                            (P, SCM, 16, 2)),
                        p23x[:])
                return A, p45

            # ---- per-super-chunk back: A -> C -> out rows ----
            def back(sc, A, p45):
                O = work.tile([P, SCM, 10], F32, tag="O")
                Ds = work.tile([P, SCM, 10, 16], F16, tag="Ds")
                Af = A[:].rearrange("p i b a -> p i (b a)")
                # DMA XBAR transpose: at2[p, 2i+k, n] = A[n-th row, 128k+p]
                at2 = atp.tile([P, 2 * SCM, P], F16, tag="at")
                for h in range(2):
                    nc.sync.dma_start_transpose(
                        out=at2[:, h * SCM:(h + 1) * SCM, :],
                        in_=Af[:, h * 4:(h + 1) * 4, :].rearrange(
                            "p i n -> p (i n)"))
                for g in range(2):          # groups of 4 chunks
                    cpp = ps_c.tile([P, 4, 256], F32, tag="cp")
                    for j in range(4):
                        i = g * 4 + j
                        for k in range(2):
                            nc.tensor.matmul(cpp[:, j, 0:160],
                                             lhsT=at2[:, 2 * i + k, :],
                                             rhs=s2_sb[:, k, :],
                                             start=(k == 0), stop=(k == 1))
                    Cs = work.tile([P, 4, 10, 16], F16, tag=f"Cs{g}")
                    nc.scalar.copy(
                        out=Cs[:].rearrange("p j c v -> p j (c v)"),
                        in_=cpp[:, :, 0:160])
                    sl = slice(g * 4, (g + 1) * 4)
                    nc.vector.tensor_mul(
                        Ds[:, sl], Cs[:],
                        p45[:, sl, None, :].broadcast_to((P, 4, 10, 16)))
                # v-contraction as an add tree (fp16 2x until the last level)
                T1 = work.tile([P, SCM, 10, 8], F16, tag="T1")
                nc.vector.tensor_add(T1[:], Ds[:, :, :, 0:8], Ds[:, :, :, 8:16])
                T2 = work.tile([P, SCM, 10, 4], F16, tag="T2")
                nc.vector.tensor_add(T2[:], T1[:, :, :, 0:4], T1[:, :, :, 4:8])
                T3 = work.tile([P, SCM, 10, 2], F16, tag="T3")
                nc.vector.tensor_add(T3[:], T2[:, :, :, 0:2], T2[:, :, :, 2:4])
                nc.vector.tensor_add(O[:], T3[:, :, :, 0], T3[:, :, :, 1])
                nc.sync.dma_start(
                    out=od[:].rearrange("p (i c) -> p i c", i=M)[
                        :, sc * SCM:(sc + 1) * SCM, :],
                    in_=O[:])

            A0, p450 = front(0)
            A1, p451 = front(1)
            back(0, A0, p450)
            back(1, A1, p451)
    nc.compile()
    return nc


_CACHE = {}


def _host_prep(x, cuts, leaf_score, temperature):
    x = np.ascontiguousarray(np.asarray(x, dtype=np.float32))
    cuts = np.asarray(cuts, dtype=np.float32)
    leaf_score = np.asarray(leaf_score, dtype=np.float32)
    invt = 1.0 / float(np.asarray(temperature).reshape(-1)[0])

    sc = np.sort(cuts, axis=1)
    bias = np.cumsum(np.concatenate([np.zeros((6, 1), np.float32), -sc],
                                    axis=1, dtype=np.float32), axis=1)  # [6,4]
    W = np.arange(1.0, 5.0, dtype=np.float32)
    w2 = np.tile(((W - 1.0) * invt)[None, :], (6, 1))                   # [6,4]
    bt = bias * invt                                                    # [6,4]
    cst = np.ascontiguousarray(np.broadcast_to(
        np.stack([w2, bt]).reshape(1, 48), (P, 48)).astype(np.float32))
    # S2[w,(c,v)] = leaf_score[w*16+v, c], then permute rows a*16+b -> b*16+a
    # to match the device A[i, b, a] memory order.
    s2 = (leaf_score.reshape(256, 16, 10).transpose(0, 2, 1)
          .reshape(16, 16, 160).transpose(1, 0, 2).reshape(256, 160))
    s2 = np.ascontiguousarray(s2.astype(np.float16))

    xs = x.reshape(N_CORES, P, M * 6)
    in_maps = [{"x": xs[i], "cst": cst, "s2": s2} for i in range(N_CORES)]
    return invt, in_maps


def kernel(x, cuts, leaf_score, temperature):
    invt, in_maps = _host_prep(x, cuts, leaf_score, temperature)
    key = ("nc", float(invt))
    if key not in _CACHE:
        _CACHE[key] = _build_nc(invt)
        _CACHE["nc"] = _CACHE[key]
    nc = _CACHE[key]
    res = run_bass_kernel_spmd(nc, in_maps, list(range(N_CORES))).results
    out = np.concatenate([r["o"].reshape(BC, 10) for r in res], axis=0)
    return out.astype(np.float32)
